# revision 25
# baseline (speedup 1.0000x reference)
"""Trainium2 Bass kernel for nn_ErrorAwareEdgeLoss.

reference:  cost[b,e] = sum_{p,q} P[b,i_e,p] * d_error[p,q] * P[b,j_e,q]
            result    = mean_{b,e} cost[b,e]

The edge pairs only enter through the count matrix
    C2[l1,l2] = C + C^T,  C[l1,l2] = #edges with (i_e,j_e) == (l1,l2),
and since d_error is symmetric the result collapses to
    result = <d_error, sum_b Q_b^T C2 Q_b> / (2*B*E),
with Q_b = P[b,:64,:].  C2 has small integer entries (max ~6): exact in
bf16/fp8.

Device work per core (256 batches, data-parallel over batch):
  - HWDGE DMA of host-packed Q groups into [128, ncols] tiles
    (two batches stacked on the 128 partitions).
  - Y = blockdiag(C2,C2) @ Q: one K=128 matmul per 512-wide slab
    (the zero off-diagonal blocks cost nothing: matmul time = N cols).
  - PSUM->SBUF casts of Y, slab-granular, alternating DVE / ACT engines.
  - R += Q_pair^T @ Y_pair (K=128 = 2 batches, N=128) accumulated in
    PSUM f32; R-matmuls run one group behind Y so the PE never waits
    on the casts.
  - write per-core R (128x128 f32) to DRAM.
Host: R_total = sum_c R_c ; result = <d_error, R_total> / (2*B*E*S^2).
"""

import sys

_TRN_REPO = "/opt/trn_rl_repo"
if _TRN_REPO not in sys.path:
    sys.path.insert(0, _TRN_REPO)

import numpy as np
import ml_dtypes

B, L, H = 2048, 64, 128     # batch, logical qubits, physical dim
E = 512                     # number of circuit edges
N_CORES = 8
BPC = B // N_CORES          # 256 batches per core
SLAB = 512                  # matmul moving-operand width (= 4 pair-blocks)
NSLABS = BPC * L * H // (128 * SLAB)   # 32 slabs of 512 cols per core

# Q dtype: "bf16" (exact) or "fp8" (halves DMA traffic; rel err ~1.5e-3)
Q_DTYPE = "fp8"
Q_SCALE = 64.0 if Q_DTYPE == "fp8" else 1.0
# Y cast output dtype; "fp8" keeps all matmuls uniform-dtype (rel ~2.2e-3)
Y_DTYPE = "fp8"
# group sizes in 512-col slabs (sum must be NSLABS). Small leading groups
# start the PE early; bigger trailing groups amortize per-DMA overhead.
GROUP_SLABS = [2, 2, 6, 6, 8, 8]
# engine that issues each group's HWDGE load (SP except where noted: the
# second group goes out on DVE so the two head issues generate in parallel)
DMA_ENGINES = ["sync", "scalar", "sync", "sync", "sync", "sync"]
# engine for each Y-slab PSUM->SBUF cast, round-robin: v=DVE, s=ACT, g=Pool
CAST_PATTERN = "vs"
# number of 512-col PE warmup matmuls issued before the first real slab
# (ramps the PE clock out of its low p-state while the first DMA lands)
WARMUP_MMS = 2

_CACHE = {}


def _np_qdt():
    return ml_dtypes.bfloat16 if Q_DTYPE == "bf16" else ml_dtypes.float8_e4m3fn


def _build():
    import concourse.tile as tile
    from concourse import bacc, mybir

    f32 = mybir.dt.float32
    bf16 = mybir.dt.bfloat16
    qdt = bf16 if Q_DTYPE == "bf16" else mybir.dt.float8e4
    ydt = bf16 if Y_DTYPE == "bf16" else mybir.dt.float8e4

    assert sum(GROUP_SLABS) == NSLABS

    nc = bacc.Bacc(None)
    # host-packed shard: row p holds, concatenated over (slab, pair-block),
    # Q[batch, p%64, :] for batch = 2*(col block) + p//64 — every group
    # load is a plain 2D DMA with a contiguous run per partition.
    pq = nc.dram_tensor("pq", [128, NSLABS * SLAB], qdt, kind="ExternalInput")
    # full block-diag(C2, C2), packed on host: one DMA, no memset needed
    cs = nc.dram_tensor("cs", [128, 128], qdt, kind="ExternalInput")
    r_out = nc.dram_tensor("r_out", [H, H], f32, kind="ExternalOutput")

    with tile.TileContext(nc) as tc:
        with (
            tc.tile_pool(name="singles", bufs=1) as singles,
            tc.tile_pool(name="qpool", bufs=6) as qpool,
            tc.tile_pool(name="ypool", bufs=6) as ypool,
            tc.tile_pool(name="yps", bufs=3, space="PSUM") as yps,
            tc.tile_pool(name="rps", bufs=1, space="PSUM") as rps,
        ):
            r_psum = rps.tile([128, H], f32)

            # warmups first: DVE memset is ready right after engine init, so
            # the PE starts its p-state ramp while the first DMAs are still
            # in flight.
            if WARMUP_MMS:
                warm = singles.tile([128, SLAB], qdt)
                nc.gpsimd.memset(warm[:, :], 0)
                wps = rps.tile([128, SLAB], f32)
                for _ in range(WARMUP_MMS):
                    nc.tensor.matmul(
                        wps[:, :], lhsT=warm[:, 0:128], rhs=warm[:, :],
                        start=True, stop=True, skip_group_check=True,
                    )

            # csbd load off the SP queue (ACT issues it) so the group-0 load
            # and the csbd load generate their descriptors concurrently.
            csbd = singles.tile([128, 128], qdt)
            nc.scalar.dma_start(out=csbd[:, :], in_=cs[:, :])

            _flags = {"first": True, "slab": 0}

            def emit_group(gi, c0, k):
                npairs = 4 * k
                qbf = qpool.tile([128, npairs, H], qdt)
                dma_eng = getattr(nc, DMA_ENGINES[gi])
                dma_eng.dma_start(out=qbf[:, :, :], in_=pq[:, c0 : c0 + k * SLAB])
                ybf = ypool.tile([128, npairs, H], ydt)
                assert k % 2 == 0, "groups must be an even number of slabs"
                for c in range(k // 2):
                    # 2-slab (1024-col) PSUM chunk: two matmuls, one cast
                    yy = yps.tile([128, 2 * SLAB], f32)
                    for h in range(2):
                        s = 2 * c + h
                        nc.tensor.matmul(
                            yy[:, h * SLAB : (h + 1) * SLAB],
                            lhsT=csbd[:, :], rhs=qbf[:, 4 * s : 4 * s + 4, :],
                            start=True, stop=True, skip_group_check=True,
                        )
                    dst = ybf[:, 8 * c : 8 * c + 8, :]
                    eng = CAST_PATTERN[_flags["slab"] % len(CAST_PATTERN)]
                    _flags["slab"] += 1
                    if eng == "v":
                        nc.vector.tensor_copy(dst, yy[:, :])
                    else:
                        nc.scalar.copy(dst, yy[:, :])
                return qbf, ybf, npairs

            def emit_r(qbf, ybf, npairs, is_last_group):
                for pp in range(npairs):
                    first = _flags["first"]
                    _flags["first"] = False
                    last = is_last_group and pp == npairs - 1
                    nc.tensor.matmul(
                        r_psum[:, :],
                        lhsT=qbf[:, pp, :],
                        rhs=ybf[:, pp, :],
                        start=first, stop=last, skip_group_check=True,
                    )

            prev = None
            c0 = 0
            for gi, k in enumerate(GROUP_SLABS):
                cur = emit_group(gi, c0, k)
                c0 += k * SLAB
                if prev is not None:
                    emit_r(*prev, is_last_group=False)
                prev = cur
            emit_r(*prev, is_last_group=True)

            rsb = singles.tile([128, H], f32)
            nc.vector.tensor_copy(rsb[:, :], r_psum[:, :])
            nc.sync.dma_start(out=r_out[:, :], in_=rsb[:, :])

    nc.compile()
    return nc


def get_nc():
    key = ("nc", Q_DTYPE, Y_DTYPE, tuple(GROUP_SLABS), WARMUP_MMS, CAST_PATTERN,
           tuple(DMA_ENGINES))
    if key not in _CACHE:
        _CACHE[key] = _build()
    return _CACHE[key]


def make_count_matrix(circuit_edge_pairs):
    pairs = np.asarray(circuit_edge_pairs).astype(np.int64)
    C = np.zeros((L, L), np.float64)
    np.add.at(C, (pairs[:, 0], pairs[:, 1]), 1.0)
    C2 = C + C.T
    bd = np.zeros((128, 128), np.float64)
    bd[:L, :L] = C2
    bd[L:, L:] = C2
    out = bd.astype(_np_qdt())
    assert np.array_equal(out.astype(np.float64), bd), "C2 not exact in Q dtype"
    return out


def pack_shard(Q):
    """(256, 64, 128) f32 -> (128, 16384): partition p, col j*128+h holds
    Q[2*j + p//64, p%64, h] * Q_SCALE (pair-blocked along columns)."""
    arr = (Q * Q_SCALE).reshape(BPC // 2, 2, L, H)
    return np.ascontiguousarray(
        arr.transpose(1, 2, 0, 3).reshape(128, NSLABS * SLAB)
    ).astype(_np_qdt())


def make_in_maps(P, circuit_edge_pairs):
    P = np.asarray(P)
    csb = make_count_matrix(circuit_edge_pairs)
    in_maps = []
    for c in range(N_CORES):
        shard = np.ascontiguousarray(
            P[c * BPC : (c + 1) * BPC, :L, :], dtype=np.float32
        )
        in_maps.append({"pq": pack_shard(shard), "cs": csb})
    return in_maps


def reduce_results(per_core_r, d_error):
    R = np.zeros((H, H), np.float64)
    for r in per_core_r:
        R += np.asarray(r).astype(np.float64)
    out = (np.asarray(d_error).astype(np.float64) * R).sum() / (
        2.0 * B * E * Q_SCALE * Q_SCALE
    )
    return np.array(out, dtype=np.float32)


def run_spmd(P, circuit_edge_pairs, **kwargs):
    """Run on the 8 NeuronCores; returns (list of per-core R, BassKernelResults)."""
    from concourse.bass_utils import run_bass_kernel_spmd

    nc = get_nc()
    in_maps = make_in_maps(P, circuit_edge_pairs)
    res = run_bass_kernel_spmd(nc, in_maps, core_ids=list(range(N_CORES)), **kwargs)
    per_core_r = [res.results[c]["r_out"] for c in range(N_CORES)]
    return per_core_r, res


def kernel(P, d_error, circuit_edge_pairs, num_logical):
    assert int(num_logical) == L
    per_core_r, _ = run_spmd(P, circuit_edge_pairs)
    return reduce_results(per_core_r, d_error)


# revision 27
# speedup vs baseline: 1.0588x; 1.0588x over previous
"""Trainium2 Bass kernel for nn_ErrorAwareEdgeLoss.

reference:  cost[b,e] = sum_{p,q} P[b,i_e,p] * d_error[p,q] * P[b,j_e,q]
            result    = mean_{b,e} cost[b,e]

The edge pairs only enter through the count matrix
    C2[l1,l2] = C + C^T,  C[l1,l2] = #edges with (i_e,j_e) == (l1,l2),
and since d_error is symmetric the result collapses to
    result = <d_error, sum_b Q_b^T C2 Q_b> / (2*B*E),
with Q_b = P[b,:64,:].  C2 has small integer entries (max ~6): exact in
bf16/fp8.

Device work per core (256 batches, data-parallel over batch):
  - HWDGE DMA of host-packed Q groups into [128, ncols] tiles
    (two batches stacked on the 128 partitions).
  - Y = blockdiag(C2,C2) @ Q: one K=128 matmul per 512-wide slab
    (the zero off-diagonal blocks cost nothing: matmul time = N cols).
  - PSUM->SBUF casts of Y, slab-granular, alternating DVE / ACT engines.
  - R += Q_pair^T @ Y_pair (K=128 = 2 batches, N=128) accumulated in
    PSUM f32; R-matmuls run one group behind Y so the PE never waits
    on the casts.
  - write per-core R (128x128 f32) to DRAM.
Host: R_total = sum_c R_c ; result = <d_error, R_total> / (2*B*E*S^2).
"""

import sys

_TRN_REPO = "/opt/trn_rl_repo"
if _TRN_REPO not in sys.path:
    sys.path.insert(0, _TRN_REPO)

import numpy as np
import ml_dtypes

B, L, H = 2048, 64, 128     # batch, logical qubits, physical dim
E = 512                     # number of circuit edges
N_CORES = 8
BPC = B // N_CORES          # 256 batches per core
SLAB = 512                  # matmul moving-operand width (= 4 pair-blocks)
NSLABS = BPC * L * H // (128 * SLAB)   # 32 slabs of 512 cols per core

# Q dtype: "bf16" (exact) or "fp8" (halves DMA traffic; rel err ~1.5e-3)
Q_DTYPE = "fp8"
Q_SCALE = 64.0 if Q_DTYPE == "fp8" else 1.0
# Y cast output dtype; "fp8" keeps all matmuls uniform-dtype (rel ~2.2e-3)
Y_DTYPE = "fp8"
# group sizes in 512-col slabs (sum must be NSLABS). Small leading groups
# start the PE early; bigger trailing groups amortize per-DMA overhead.
GROUP_SLABS = [2, 4, 6, 6, 6, 8]
# engine that issues each group's HWDGE load (SP or Activation only)
DMA_ENGINES = ["sync", "sync", "sync", "sync", "sync", "sync"]
# engine for each Y-slab PSUM->SBUF cast, round-robin: v=DVE, s=ACT, g=Pool
CAST_PATTERN = "vs"
# number of 512-col PE warmup matmuls issued before the first real slab
# (ramps the PE clock out of its low p-state while the first DMA lands)
WARMUP_MMS = 2

_CACHE = {}


def _np_qdt():
    return ml_dtypes.bfloat16 if Q_DTYPE == "bf16" else ml_dtypes.float8_e4m3fn


def _build():
    import concourse.tile as tile
    from concourse import bacc, mybir

    f32 = mybir.dt.float32
    bf16 = mybir.dt.bfloat16
    qdt = bf16 if Q_DTYPE == "bf16" else mybir.dt.float8e4
    ydt = bf16 if Y_DTYPE == "bf16" else mybir.dt.float8e4

    assert sum(GROUP_SLABS) == NSLABS

    nc = bacc.Bacc(None)
    # host-packed shard: row p holds, concatenated over (slab, pair-block),
    # Q[batch, p%64, :] for batch = 2*(col block) + p//64 — every group
    # load is a plain 2D DMA with a contiguous run per partition.
    pq = nc.dram_tensor("pq", [128, NSLABS * SLAB], qdt, kind="ExternalInput")
    # full block-diag(C2, C2), packed on host: one DMA, no memset needed
    cs = nc.dram_tensor("cs", [128, 128], qdt, kind="ExternalInput")
    r_out = nc.dram_tensor("r_out", [H, H], f32, kind="ExternalOutput")

    with tile.TileContext(nc) as tc:
        with (
            tc.tile_pool(name="singles", bufs=1) as singles,
            tc.tile_pool(name="qpool", bufs=6) as qpool,
            tc.tile_pool(name="ypool", bufs=6) as ypool,
            tc.tile_pool(name="yps", bufs=3, space="PSUM") as yps,
            tc.tile_pool(name="rps", bufs=1, space="PSUM") as rps,
        ):
            # csbd load off the SP queue (ACT issues it) so the group-0 load
            # and the csbd load generate their descriptors concurrently.
            csbd = singles.tile([128, 128], qdt)
            nc.scalar.dma_start(out=csbd[:, :], in_=cs[:, :])

            r_psum = rps.tile([128, H], f32)

            # warmup matmuls on a zeroed tile start the PE p-state ramp
            # while the first group's DMA is still in flight.
            if WARMUP_MMS:
                warm = singles.tile([128, SLAB], qdt)
                nc.gpsimd.memset(warm[:, :], 0)
                wps = rps.tile([128, SLAB], f32)
                for _ in range(WARMUP_MMS):
                    nc.tensor.matmul(
                        wps[:, :], lhsT=warm[:, 0:128], rhs=warm[:, :],
                        start=True, stop=True, skip_group_check=True,
                    )

            _flags = {"first": True, "slab": 0}

            def emit_group(gi, c0, k):
                npairs = 4 * k
                qbf = qpool.tile([128, npairs, H], qdt)
                dma_eng = getattr(nc, DMA_ENGINES[gi])
                dma_eng.dma_start(out=qbf[:, :, :], in_=pq[:, c0 : c0 + k * SLAB])
                ybf = ypool.tile([128, npairs, H], ydt)
                assert k % 2 == 0, "groups must be an even number of slabs"
                for c in range(k // 2):
                    # 2-slab (1024-col) PSUM chunk: two matmuls, one cast
                    yy = yps.tile([128, 2 * SLAB], f32)
                    for h in range(2):
                        s = 2 * c + h
                        nc.tensor.matmul(
                            yy[:, h * SLAB : (h + 1) * SLAB],
                            lhsT=csbd[:, :], rhs=qbf[:, 4 * s : 4 * s + 4, :],
                            start=True, stop=True, skip_group_check=True,
                        )
                    dst = ybf[:, 8 * c : 8 * c + 8, :]
                    eng = CAST_PATTERN[_flags["slab"] % len(CAST_PATTERN)]
                    _flags["slab"] += 1
                    if eng == "v":
                        nc.vector.tensor_copy(dst, yy[:, :])
                    else:
                        nc.scalar.copy(dst, yy[:, :])
                return qbf, ybf, npairs

            def emit_r(qbf, ybf, npairs, is_last_group):
                for pp in range(npairs):
                    first = _flags["first"]
                    _flags["first"] = False
                    last = is_last_group and pp == npairs - 1
                    nc.tensor.matmul(
                        r_psum[:, :],
                        lhsT=qbf[:, pp, :],
                        rhs=ybf[:, pp, :],
                        start=first, stop=last, skip_group_check=True,
                    )

            prev = None
            c0 = 0
            for gi, k in enumerate(GROUP_SLABS):
                cur = emit_group(gi, c0, k)
                c0 += k * SLAB
                if prev is not None:
                    emit_r(*prev, is_last_group=False)
                prev = cur
            emit_r(*prev, is_last_group=True)

            rsb = singles.tile([128, H], f32)
            nc.vector.tensor_copy(rsb[:, :], r_psum[:, :])
            nc.sync.dma_start(out=r_out[:, :], in_=rsb[:, :])

    nc.compile()
    return nc


def get_nc():
    key = ("nc", Q_DTYPE, Y_DTYPE, tuple(GROUP_SLABS), WARMUP_MMS, CAST_PATTERN,
           tuple(DMA_ENGINES))
    if key not in _CACHE:
        _CACHE[key] = _build()
    return _CACHE[key]


def make_count_matrix(circuit_edge_pairs):
    pairs = np.asarray(circuit_edge_pairs).astype(np.int64)
    C = np.zeros((L, L), np.float64)
    np.add.at(C, (pairs[:, 0], pairs[:, 1]), 1.0)
    C2 = C + C.T
    bd = np.zeros((128, 128), np.float64)
    bd[:L, :L] = C2
    bd[L:, L:] = C2
    out = bd.astype(_np_qdt())
    assert np.array_equal(out.astype(np.float64), bd), "C2 not exact in Q dtype"
    return out


def pack_shard(Q):
    """(256, 64, 128) f32 -> (128, 16384): partition p, col j*128+h holds
    Q[2*j + p//64, p%64, h] * Q_SCALE (pair-blocked along columns)."""
    arr = (Q * Q_SCALE).reshape(BPC // 2, 2, L, H)
    return np.ascontiguousarray(
        arr.transpose(1, 2, 0, 3).reshape(128, NSLABS * SLAB)
    ).astype(_np_qdt())


def make_in_maps(P, circuit_edge_pairs):
    P = np.asarray(P)
    csb = make_count_matrix(circuit_edge_pairs)
    in_maps = []
    for c in range(N_CORES):
        shard = np.ascontiguousarray(
            P[c * BPC : (c + 1) * BPC, :L, :], dtype=np.float32
        )
        in_maps.append({"pq": pack_shard(shard), "cs": csb})
    return in_maps


def reduce_results(per_core_r, d_error):
    R = np.zeros((H, H), np.float64)
    for r in per_core_r:
        R += np.asarray(r).astype(np.float64)
    out = (np.asarray(d_error).astype(np.float64) * R).sum() / (
        2.0 * B * E * Q_SCALE * Q_SCALE
    )
    return np.array(out, dtype=np.float32)


def run_spmd(P, circuit_edge_pairs, **kwargs):
    """Run on the 8 NeuronCores; returns (list of per-core R, BassKernelResults)."""
    from concourse.bass_utils import run_bass_kernel_spmd

    nc = get_nc()
    in_maps = make_in_maps(P, circuit_edge_pairs)
    res = run_bass_kernel_spmd(nc, in_maps, core_ids=list(range(N_CORES)), **kwargs)
    per_core_r = [res.results[c]["r_out"] for c in range(N_CORES)]
    return per_core_r, res


def kernel(P, d_error, circuit_edge_pairs, num_logical):
    assert int(num_logical) == L
    per_core_r, _ = run_spmd(P, circuit_edge_pairs)
    return reduce_results(per_core_r, d_error)
